# revision 20
# baseline (speedup 1.0000x reference)
"""CapsLayer2D dynamic-routing kernel for 8x TRN2 NeuronCores.

Problem (hardcoded shapes):
  inputs: [B=16, R=8, C=8, I=128, DIN=16] fp32
  W:      [K=32, I=128, DIN=16, DOUT=16] fp32
  out:    [B, R, C, K, DOUT] fp32

Math (3-round dynamic routing, closed form, verified vs reference):
  U[p,k]    = res[p,k,:,:]  (I x O per position p=(b,r,c) and k)
  s0        = mean_i U_i ; v0 = squash(s0)
  t_a = U v0 ; m_a = U^T t_a ; s1 = s0 + m_a ; v1 = squash(s1)
  t_b = U (v0+v1) ; m_b = U^T t_b ; s2 = s0 + m_b ; out = squash(s2)

Sharding: batch across 8 cores (128 positions/core), W replicated.

Performance design (v5):
  - All W/X layout work (pad d 16->32, transpose to matmul operand
    layout, fp32->fp16 cast) is host-side numpy: zero device prep.
  - 4 k-groups of 8 caps. Production per group: s0 via 32 accumulating
    matmuls; res via 128 per-i matmuls (tile_position quadrants), one
    full PSUM bank per matmul (concurrent start/stop groups must not
    share a bank), strided cross-bank evictions on Act.
  - Routing on the DVE with TENSOR_TENSOR only (2x mode: fp16,
    unit-stride innermost; TRN2 has no 4x for two-stream ops, and
    tensor_reduce has no perf modes at all). Contractions are log2
    trees over sliced views, computed in place inside one scratch
    tile. The U^T t contraction reads t through a duplicated-pair
    tile t2[p,k,i,2] built by a single butterfly add (reversed-stride
    operand), keeping every operand's innermost AP packed.
  - Two groups are software-pipelined: the small tree tails run on the
    Pool engine while the DVE works on the other group, so the DVE
    stream stays gap-free.
"""

import sys

import numpy as np

sys.path.insert(0, "/opt/trn_rl_repo")

P, I, D, K, O = 128, 128, 16, 32, 16
D2 = 32  # padded d
ID = I * D  # 2048
KO = K * O  # 512
KC = 8  # k-group size
NG = K // KC  # 4 groups
GW = KC * O  # 128 group output width
GKO = 32 * KC * O  # per-group W cols: 32 chunks x (k8,o16) = 4096
N_CORES = 8
EPS = 1e-7

_PROGRAM = None


def _build_program():
    from contextlib import ExitStack

    import concourse.tile as tile
    from concourse import bacc, mybir

    F32 = mybir.dt.float32
    F16 = mybir.dt.float16
    ADD = mybir.AluOpType.add
    MULT = mybir.AluOpType.mult
    X = mybir.AxisListType.X
    SQRT = mybir.ActivationFunctionType.Sqrt

    nc = bacc.Bacc("TRN2", target_bir_lowering=False, debug=False)

    xt_d = nc.dram_tensor("xt", [128, 32 * 128], F16, kind="ExternalInput").ap()
    wr_d = nc.dram_tensor("wr", [128, NG * GKO], F16, kind="ExternalInput").ap()
    out_d = nc.dram_tensor("out", [P, KO], F32, kind="ExternalOutput").ap()

    with ExitStack() as ctx:
        tc = ctx.enter_context(tile.TileContext(nc))

        pp_s = ctx.enter_context(tc.tile_pool(name="pp_s", bufs=2, space="PSUM"))
        pp_r = ctx.enter_context(tc.tile_pool(name="pp_r", bufs=2, space="PSUM"))

        xp = ctx.enter_context(tc.tile_pool(name="xt", bufs=1))
        wp = ctx.enter_context(tc.tile_pool(name="wr", bufs=2))
        rp = ctx.enter_context(tc.tile_pool(name="res", bufs=2))
        sp = ctx.enter_context(tc.tile_pool(name="scratch", bufs=1))
        sm = ctx.enter_context(tc.tile_pool(name="small", bufs=1))

        Xt = xp.tile([128, 32 * 128], F16)
        for q in range(4):
            nc.sync.dma_start(
                Xt[:, q * 1024:(q + 1) * 1024],
                xt_d[:, q * 1024:(q + 1) * 1024],
            )

        eps_t = sm.tile([P, 1], F32, tag="eps")
        nc.vector.memset(eps_t[:], EPS)

        state = {g: {} for g in range(NG)}

        def squash(g, s_ap, v_ap, tag):
            """v = squash(s); fp32 [P, (k8,o16)]; sqrt on Act."""
            pr = g % 2
            ssq = sm.tile([P, GW], F32, tag=f"ssq{pr}")
            nc.vector.tensor_mul(ssq[:], s_ap, s_ap)
            sq = sm.tile([P, KC], F32, tag=f"sq{pr}_{tag}")
            nc.vector.tensor_reduce(
                sq[:], ssq[:].rearrange("p (k o) -> p k o", k=KC), X, ADD
            )
            a = sm.tile([P, KC], F32, tag=f"sqa{pr}")
            nc.scalar.activation(a[:], sq[:], SQRT, bias=eps_t[:])
            b = sm.tile([P, KC], F32, tag=f"sqb{pr}")
            nc.vector.scalar_tensor_tensor(b[:], sq[:], 1.0, a[:], ADD, MULT)
            r = sm.tile([P, KC], F32, tag=f"sqr{pr}")
            nc.vector.reciprocal(r[:], b[:])
            f = sm.tile([P, KC], F32, tag=f"sqf{pr}")
            nc.vector.tensor_mul(f[:], sq[:], r[:])
            nc.vector.tensor_mul(
                v_ap.rearrange("p (k o) -> p k o", k=KC),
                s_ap.rearrange("p (k o) -> p k o", k=KC),
                f[:].unsqueeze(2).broadcast_to([P, KC, O]),
            )

        def produce(g):
            st = state[g]
            pr = g % 2
            W_g = wp.tile([128, GKO], F16, tag="wg")
            for q in range(4):
                nc.sync.dma_start(
                    W_g[:, q * 1024:(q + 1) * 1024],
                    wr_d[:, g * GKO + q * 1024:g * GKO + (q + 1) * 1024],
                )

            ps0 = pp_s.tile([P, 512], F32, tag="ps0")
            for c in range(32):
                nc.tensor.matmul(
                    ps0[:, 0:GW],
                    Xt[:, c * 128:(c + 1) * 128],
                    W_g[:, c * 128:(c + 1) * 128],
                    start=(c == 0),
                    stop=(c == 31),
                )
            s0 = sm.tile([P, GW], F32, tag=f"s0_{pr}")
            nc.scalar.mul(s0[:], ps0[:, 0:GW], 1.0 / I)
            v0 = sm.tile([P, GW], F32, tag=f"v0_{pr}")
            squash(g, s0[:], v0[:], "v0")
            v0h = sm.tile([P, GW], F16, tag=f"v0h{pr}")
            nc.vector.tensor_copy(v0h[:], v0[:])

            res = rp.tile([P, KC * I * O], F16, tag="res")
            resv = res[:].rearrange("p (k i o) -> p k i o", k=KC, i=I, o=O)
            for c in range(32):
                for m in range(2):
                    prb = pp_r.tile([P, 1024], F32, tag="prb")
                    for j in (2 * m, 2 * m + 1):
                        r0 = j * 32
                        nc.tensor.matmul(
                            prb[:, (j % 2) * 512:(j % 2) * 512 + GW],
                            Xt[r0:r0 + 32, c * 128:(c + 1) * 128],
                            W_g[r0:r0 + 32, c * 128:(c + 1) * 128],
                            start=True,
                            stop=True,
                            tile_position=(r0, 0),
                        )
                    src = prb[:].rearrange("p (i x) -> p i x", i=2)[
                        :, :, 0:GW
                    ].rearrange("p i (k o) -> p i k o", k=KC)
                    dst = resv[
                        :, :, 4 * c + 2 * m:4 * c + 2 * m + 2, :
                    ].transpose([0, 2, 1, 3])
                    # groups 0/1: DVE is idle before its first work, so
                    # splitting evictions shortens the startup path
                    if g <= 1 and m == 1:
                        nc.vector.tensor_copy(dst, src)
                    else:
                        nc.scalar.copy(dst, src)
            st["res"] = res
            st["s0"] = s0
            st["v0"] = v0
            st["v0h"] = v0h

        def S_uv(g, vkey):
            """DVE: tmp = res * v (bcast over i); in-place r8 step."""
            st = state[g]
            tmp = sp.tile([P, KC * I * O], F16, tag=f"tmp{g % 2}")
            st["tmp"] = tmp
            t4 = tmp[:].rearrange("p (k i o) -> p k i o", k=KC, i=I)
            rv4 = st["res"][:].rearrange("p (k i o) -> p k i o", k=KC, i=I)
            vb4 = (
                st[vkey][:]
                .rearrange("p (k o) -> p k o", k=KC)
                .unsqueeze(2)
                .broadcast_to([P, KC, I, O])
            )
            nc.vector.tensor_mul(t4, rv4, vb4)
            nc.vector.tensor_add(
                t4[:, :, :, 0:8], t4[:, :, :, 0:8], t4[:, :, :, 8:16]
            )

        def S_otail(g):
            """Pool: in-place o-tree 8 -> 4 -> 2."""
            t4 = state[g]["tmp"][:].rearrange("p (k i o) -> p k i o", k=KC, i=I)
            nc.gpsimd.tensor_add(
                t4[:, :, :, 0:4], t4[:, :, :, 0:4], t4[:, :, :, 4:8]
            )
            nc.gpsimd.tensor_add(
                t4[:, :, :, 0:2], t4[:, :, :, 0:2], t4[:, :, :, 2:4]
            )

        def S_btf(g):
            """DVE: butterfly -> both t2 slots get the o-pair sum."""
            st = state[g]
            t2 = sp.tile([P, KC * I * 2], F16, tag=f"t2{g % 2}")
            st["t2"] = t2
            r2v = (
                state[g]["tmp"][:]
                .rearrange("p (k i o) -> p k i o", k=KC, i=I)[:, :, :, 0:2]
            )
            t2v = t2[:].rearrange("p (k i two) -> p k i two", k=KC, i=I)
            nc.vector.tensor_add(t2v, r2v, r2v[:, :, :, ::-1])

        def S_ut(g):
            """DVE: tmp = res * t2-pairs; in-place i-tree 128 -> 32."""
            st = state[g]
            tmp = sp.tile([P, KC * I * O], F16, tag=f"tmp{g % 2}")
            st["tmp"] = tmp
            t5 = tmp[:].rearrange(
                "p (k i o2 two) -> p k i o2 two", k=KC, i=I, o2=O // 2
            )
            rv5 = st["res"][:].rearrange(
                "p (k i o2 two) -> p k i o2 two", k=KC, i=I, o2=O // 2
            )
            tb5 = (
                st["t2"][:]
                .rearrange("p (k i two) -> p k i two", k=KC, i=I)
                .unsqueeze(3)
                .broadcast_to([P, KC, I, O // 2, 2])
            )
            nc.vector.tensor_mul(t5, rv5, tb5)
            t4 = tmp[:].rearrange("p (k i o) -> p k i o", k=KC, i=I)
            nc.vector.tensor_add(
                t4[:, :, 0:64, :], t4[:, :, 0:64, :], t4[:, :, 64:128, :]
            )
            nc.vector.tensor_add(
                t4[:, :, 0:32, :], t4[:, :, 0:32, :], t4[:, :, 32:64, :]
            )

        def S_itail(g):
            """Pool: in-place i-tree 32 -> 2, then m = row0 + row1."""
            st = state[g]
            t4 = st["tmp"][:].rearrange("p (k i o) -> p k i o", k=KC, i=I)
            n = 32
            while n > 2:
                h = n // 2
                nc.gpsimd.tensor_add(
                    t4[:, :, 0:h, :], t4[:, :, 0:h, :], t4[:, :, h:n, :]
                )
                n = h
            m_t = sm.tile([P, GW], F16, tag=f"m{g % 2}")
            st["m"] = m_t
            nc.gpsimd.tensor_add(
                m_t[:].rearrange("p (k o) -> p k o", k=KC),
                t4[:, :, 0, :],
                t4[:, :, 1, :],
            )

        def S_mid(g):
            """s1 = s0 + m_a; v1 = squash(s1); vsh = fp16(v0 + v1)."""
            st = state[g]
            pr = g % 2
            s1 = sm.tile([P, GW], F32, tag=f"s1_{pr}")
            nc.vector.tensor_add(s1[:], st["s0"][:], st["m"][:])
            v1 = sm.tile([P, GW], F32, tag=f"v1_{pr}")
            squash(g, s1[:], v1[:], "v1")
            vs = sm.tile([P, GW], F32, tag=f"vs{pr}")
            nc.vector.tensor_add(vs[:], st["v0"][:], v1[:])
            vsh = sm.tile([P, GW], F16, tag=f"vsh{pr}")
            nc.vector.tensor_copy(vsh[:], vs[:])
            st["vsh"] = vsh

        def S_out(g):
            """s2 = s0 + m_b; out = squash(s2); DMA."""
            st = state[g]
            pr = g % 2
            s2 = sm.tile([P, GW], F32, tag=f"s2_{pr}")
            nc.vector.tensor_add(s2[:], st["s0"][:], st["m"][:])
            outt = sm.tile([P, GW], F32, tag=f"outt{pr}")
            squash(g, s2[:], outt[:], "out")
            nc.sync.dma_start(out_d[:, g * GW:(g + 1) * GW], outt[:])

        with nc.allow_low_precision(reason="fp16 routing intermediates"):
            produce(0)
            produce(1)
            for A, B in ((0, 1), (2, 3)):
                S_uv(A, "v0h"); S_otail(A)
                S_uv(B, "v0h"); S_otail(B)
                S_btf(A); S_ut(A); S_itail(A)
                S_btf(B); S_ut(B); S_itail(B)
                S_mid(A); S_uv(A, "vsh"); S_otail(A)
                S_mid(B); S_uv(B, "vsh"); S_otail(B)
                S_btf(A); S_ut(A); S_itail(A)
                if A == 0:
                    produce(2)
                S_btf(B); S_ut(B); S_itail(B)
                if A == 0:
                    produce(3)
                S_out(A)
                S_out(B)

    nc.compile()
    return nc


def _get_program():
    global _PROGRAM
    if _PROGRAM is None:
        _PROGRAM = _build_program()
    return _PROGRAM


def _make_in_maps(inputs):
    x = np.ascontiguousarray(np.asarray(inputs["inputs"], dtype=np.float32))
    W = np.ascontiguousarray(np.asarray(inputs["W"], dtype=np.float32))
    assert x.shape == (16, 8, 8, 128, 16) and W.shape == (32, 128, 16, 16)

    # xt rows: (i%4)*32 + d, cols: (i//4)*128 + p  (d padded 16->32)
    xs = x.reshape(N_CORES, P, I, D)  # [core, p, i, d]
    xt = np.zeros((N_CORES, 4, D2, 32, P), np.float32)
    # [core, i4, d, c, p] <- [core, c, i4, d, p]
    xt[:, :, 0:D] = xs.reshape(N_CORES, P, 32, 4, D).transpose(0, 3, 4, 2, 1)
    xt = xt.reshape(N_CORES, 128, 32 * 128).astype(np.float16)

    # wr rows: (i%4)*32 + d, cols: g*4096 + (i//4)*128 + (k%8)*16 + o
    wv = W.reshape(NG, KC, 32, 4, D, O)  # [g, k8, c, i4, d, o]
    wr = np.zeros((4, D2, NG, 32, KC, O), np.float32)  # [i4, d, g, c, k8, o]
    wr[:, 0:D] = wv.transpose(3, 4, 0, 2, 1, 5)
    wr = np.ascontiguousarray(
        wr.reshape(128, NG * GKO).astype(np.float16)
    )

    return [
        {"xt": np.ascontiguousarray(xt[c]), "wr": wr} for c in range(N_CORES)
    ]


def kernel(**inputs):
    from concourse.bass_utils import run_bass_kernel_spmd

    nc = _get_program()
    in_maps = _make_in_maps(inputs)
    r = run_bass_kernel_spmd(nc, in_maps, list(range(N_CORES)))
    outs = [r.results[c]["out"].reshape(2, 8, 8, K, O) for c in range(N_CORES)]
    return np.concatenate(outs, axis=0).astype(np.float32)


# revision 25
# speedup vs baseline: 1.1657x; 1.1657x over previous
"""CapsLayer2D dynamic-routing kernel for 8x TRN2 NeuronCores.

Problem (hardcoded shapes):
  inputs: [B=16, R=8, C=8, I=128, DIN=16] fp32
  W:      [K=32, I=128, DIN=16, DOUT=16] fp32
  out:    [B, R, C, K, DOUT] fp32

Math (3-round dynamic routing, closed form, verified vs reference):
  U[p,k]    = res[p,k,:,:]  (I x O per position p=(b,r,c) and k)
  s0        = mean_i U_i ; v0 = squash(s0)
  t_a = U v0 ; m_a = U^T t_a ; s1 = s0 + m_a ; v1 = squash(s1)
  t_b = U (v0+v1) ; m_b = U^T t_b ; s2 = s0 + m_b ; out = squash(s2)

Sharding: batch across 8 cores (128 positions/core), W replicated.

Performance design (v5):
  - All W/X layout work (pad d 16->32, transpose to matmul operand
    layout, fp32->fp16 cast) is host-side numpy: zero device prep.
  - 4 k-groups of 8 caps. Production per group: s0 via 32 accumulating
    matmuls; res via 128 per-i matmuls (tile_position quadrants), one
    full PSUM bank per matmul (concurrent start/stop groups must not
    share a bank), strided cross-bank evictions on Act.
  - Routing on the DVE with TENSOR_TENSOR only (2x mode: fp16,
    unit-stride innermost; TRN2 has no 4x for two-stream ops, and
    tensor_reduce has no perf modes at all). Contractions are log2
    trees over sliced views, computed in place inside one scratch
    tile. The U^T t contraction reads t through a duplicated-pair
    tile t2[p,k,i,2] built by a single butterfly add (reversed-stride
    operand), keeping every operand's innermost AP packed.
  - Two groups are software-pipelined: the small tree tails run on the
    Pool engine while the DVE works on the other group, so the DVE
    stream stays gap-free.
"""

import sys

import numpy as np

sys.path.insert(0, "/opt/trn_rl_repo")

P, I, D, K, O = 128, 128, 16, 32, 16
D2 = 32  # padded d
ID = I * D  # 2048
KO = K * O  # 512
KC = 8  # k-group size
NG = K // KC  # 4 groups
GW = KC * O  # 128 group output width
GKO = 32 * KC * O  # per-group W cols: 32 chunks x (k8,o16) = 4096
N_CORES = 8
EPS = 1e-7

_PROGRAM = None


def _build_program():
    from contextlib import ExitStack

    import concourse.tile as tile
    from concourse import bacc, mybir

    F32 = mybir.dt.float32
    F16 = mybir.dt.float16
    ADD = mybir.AluOpType.add
    MULT = mybir.AluOpType.mult
    X = mybir.AxisListType.X
    SQRT = mybir.ActivationFunctionType.Sqrt

    nc = bacc.Bacc("TRN2", target_bir_lowering=False, debug=False)

    xt_d = nc.dram_tensor("xt", [128, 32 * 128], F16, kind="ExternalInput").ap()
    wr_d = nc.dram_tensor("wr", [128, NG * GKO], F16, kind="ExternalInput").ap()
    out_d = nc.dram_tensor("out", [P, KO], F32, kind="ExternalOutput").ap()

    with ExitStack() as ctx:
        tc = ctx.enter_context(tile.TileContext(nc))

        pp_s = ctx.enter_context(tc.tile_pool(name="pp_s", bufs=2, space="PSUM"))
        pp_r = ctx.enter_context(tc.tile_pool(name="pp_r", bufs=2, space="PSUM"))

        xp = ctx.enter_context(tc.tile_pool(name="xt", bufs=1))
        wp = ctx.enter_context(tc.tile_pool(name="wr", bufs=1))
        rp = ctx.enter_context(tc.tile_pool(name="res", bufs=3))
        sp = ctx.enter_context(tc.tile_pool(name="scratch", bufs=1))
        sm = ctx.enter_context(tc.tile_pool(name="small", bufs=1))

        Xt = xp.tile([128, 32 * 128], F16)
        for q in range(4):
            nc.sync.dma_start(
                Xt[:, q * 1024:(q + 1) * 1024],
                xt_d[:, q * 1024:(q + 1) * 1024],
            )

        eps_t = sm.tile([P, 1], F32, tag="eps")
        nc.vector.memset(eps_t[:], EPS)

        state = {g: {} for g in range(NG)}

        def squash(g, s_ap, v_ap, tag):
            """v = squash(s); fp32 [P, (k8,o16)]; sqrt on Act."""
            pr = g % 2
            ssq = sm.tile([P, GW], F32, tag=f"ssq{pr}")
            nc.vector.tensor_mul(ssq[:], s_ap, s_ap)
            sq = sm.tile([P, KC], F32, tag=f"sq{pr}_{tag}")
            nc.vector.tensor_reduce(
                sq[:], ssq[:].rearrange("p (k o) -> p k o", k=KC), X, ADD
            )
            a = sm.tile([P, KC], F32, tag=f"sqa{pr}")
            nc.scalar.activation(a[:], sq[:], SQRT, bias=eps_t[:])
            b = sm.tile([P, KC], F32, tag=f"sqb{pr}")
            nc.vector.scalar_tensor_tensor(b[:], sq[:], 1.0, a[:], ADD, MULT)
            r = sm.tile([P, KC], F32, tag=f"sqr{pr}")
            nc.vector.reciprocal(r[:], b[:])
            f = sm.tile([P, KC], F32, tag=f"sqf{pr}")
            nc.vector.tensor_mul(f[:], sq[:], r[:])
            nc.vector.tensor_mul(
                v_ap.rearrange("p (k o) -> p k o", k=KC),
                s_ap.rearrange("p (k o) -> p k o", k=KC),
                f[:].unsqueeze(2).broadcast_to([P, KC, O]),
            )

        def produce(g):
            st = state[g]
            pr = g % 2
            W_g = wp.tile([128, GKO], F16, tag="wg")
            for q in range(4):
                nc.sync.dma_start(
                    W_g[:, q * 1024:(q + 1) * 1024],
                    wr_d[:, g * GKO + q * 1024:g * GKO + (q + 1) * 1024],
                )

            res = rp.tile([P, KC * I * O], F16, tag="res")
            resv = res[:].rearrange("p (k i o) -> p k i o", k=KC, i=I, o=O)
            for c in range(32):
                for m in range(2):
                    prb = pp_r.tile([P, 1024], F32, tag="prb")
                    for j in (2 * m, 2 * m + 1):
                        r0 = j * 32
                        nc.tensor.matmul(
                            prb[:, (j % 2) * 512:(j % 2) * 512 + GW],
                            Xt[r0:r0 + 32, c * 128:(c + 1) * 128],
                            W_g[r0:r0 + 32, c * 128:(c + 1) * 128],
                            start=True,
                            stop=True,
                            tile_position=(r0, 0),
                        )
                    src = prb[:].rearrange("p (i x) -> p i x", i=2)[
                        :, :, 0:GW
                    ].rearrange("p i (k o) -> p i k o", k=KC)
                    dst = resv[
                        :, :, 4 * c + 2 * m:4 * c + 2 * m + 2, :
                    ].transpose([0, 2, 1, 3])
                    # groups 0/1: DVE is idle before its first work, so
                    # splitting evictions shortens the startup path
                    if g <= 1 and m == 1:
                        nc.vector.tensor_copy(dst, src)
                    else:
                        nc.scalar.copy(dst, src)

            # s0 after res: the PE runs res matmuls first so the DVE's
            # first evictions/routing unblock as early as possible
            ps0 = pp_s.tile([P, 512], F32, tag="ps0")
            for c in range(32):
                nc.tensor.matmul(
                    ps0[:, 0:GW],
                    Xt[:, c * 128:(c + 1) * 128],
                    W_g[:, c * 128:(c + 1) * 128],
                    start=(c == 0),
                    stop=(c == 31),
                )
            s0 = sm.tile([P, GW], F32, tag=f"s0_{pr}")
            nc.scalar.mul(s0[:], ps0[:, 0:GW], 1.0 / I)
            v0 = sm.tile([P, GW], F32, tag=f"v0_{pr}")
            squash(g, s0[:], v0[:], "v0")
            v0h = sm.tile([P, GW], F16, tag=f"v0h{pr}")
            nc.vector.tensor_copy(v0h[:], v0[:])

            st["res"] = res
            st["s0"] = s0
            st["v0"] = v0
            st["v0h"] = v0h

        def S_uv(g, vkey):
            """DVE: tmp = res * v (bcast over i); in-place r8 step."""
            st = state[g]
            tmp = sp.tile([P, KC * I * O], F16, tag=f"tmp{g % 2}")
            st["tmp"] = tmp
            t4 = tmp[:].rearrange("p (k i o) -> p k i o", k=KC, i=I)
            rv4 = st["res"][:].rearrange("p (k i o) -> p k i o", k=KC, i=I)
            vb4 = (
                st[vkey][:]
                .rearrange("p (k o) -> p k o", k=KC)
                .unsqueeze(2)
                .broadcast_to([P, KC, I, O])
            )
            nc.vector.tensor_mul(t4, rv4, vb4)
            nc.vector.tensor_add(
                t4[:, :, :, 0:8], t4[:, :, :, 0:8], t4[:, :, :, 8:16]
            )
            nc.vector.tensor_add(
                t4[:, :, :, 0:4], t4[:, :, :, 0:4], t4[:, :, :, 4:8]
            )

        def S_otail(g):
            """Pool: in-place o-tree 4 -> 2 (Pool runs ~4ns/el on these
            2-dim APs; only the small tail fits its speed)."""
            t4 = state[g]["tmp"][:].rearrange("p (k i o) -> p k i o", k=KC, i=I)
            nc.gpsimd.tensor_add(
                t4[:, :, :, 0:2], t4[:, :, :, 0:2], t4[:, :, :, 2:4]
            )

        def S_btf(g):
            """DVE: butterfly -> both t2 slots get the o-pair sum."""
            st = state[g]
            t2 = sp.tile([P, KC * I * 2], F16, tag=f"t2{g % 2}")
            st["t2"] = t2
            r2v = (
                state[g]["tmp"][:]
                .rearrange("p (k i o) -> p k i o", k=KC, i=I)[:, :, :, 0:2]
            )
            t2v = t2[:].rearrange("p (k i two) -> p k i two", k=KC, i=I)
            nc.vector.tensor_add(t2v, r2v, r2v[:, :, :, ::-1])

        def S_ut(g):
            """DVE: tmp = res * t2-pairs; in-place i-tree 128 -> 32."""
            st = state[g]
            tmp = sp.tile([P, KC * I * O], F16, tag=f"tmp{g % 2}")
            st["tmp"] = tmp
            t5 = tmp[:].rearrange(
                "p (k i o2 two) -> p k i o2 two", k=KC, i=I, o2=O // 2
            )
            rv5 = st["res"][:].rearrange(
                "p (k i o2 two) -> p k i o2 two", k=KC, i=I, o2=O // 2
            )
            tb5 = (
                st["t2"][:]
                .rearrange("p (k i two) -> p k i two", k=KC, i=I)
                .unsqueeze(3)
                .broadcast_to([P, KC, I, O // 2, 2])
            )
            nc.vector.tensor_mul(t5, rv5, tb5)
            t4 = tmp[:].rearrange("p (k i o) -> p k i o", k=KC, i=I)
            nc.vector.tensor_add(
                t4[:, :, 0:64, :], t4[:, :, 0:64, :], t4[:, :, 64:128, :]
            )
            nc.vector.tensor_add(
                t4[:, :, 0:32, :], t4[:, :, 0:32, :], t4[:, :, 32:64, :]
            )
            nc.vector.tensor_add(
                t4[:, :, 0:16, :], t4[:, :, 0:16, :], t4[:, :, 16:32, :]
            )

        def S_itail(g):
            """Pool: in-place i-tree 16 -> 2, then m = row0 + row1."""
            st = state[g]
            t4 = st["tmp"][:].rearrange("p (k i o) -> p k i o", k=KC, i=I)
            n = 16
            while n > 2:
                h = n // 2
                nc.gpsimd.tensor_add(
                    t4[:, :, 0:h, :], t4[:, :, 0:h, :], t4[:, :, h:n, :]
                )
                n = h
            m_t = sm.tile([P, GW], F16, tag=f"m{g % 2}")
            st["m"] = m_t
            nc.gpsimd.tensor_add(
                m_t[:].rearrange("p (k o) -> p k o", k=KC),
                t4[:, :, 0, :],
                t4[:, :, 1, :],
            )

        def S_mid(g):
            """s1 = s0 + m_a; v1 = squash(s1); vsh = fp16(v0 + v1)."""
            st = state[g]
            pr = g % 2
            s1 = sm.tile([P, GW], F32, tag=f"s1_{pr}")
            nc.vector.tensor_add(s1[:], st["s0"][:], st["m"][:])
            v1 = sm.tile([P, GW], F32, tag=f"v1_{pr}")
            squash(g, s1[:], v1[:], "v1")
            vs = sm.tile([P, GW], F32, tag=f"vs{pr}")
            nc.vector.tensor_add(vs[:], st["v0"][:], v1[:])
            vsh = sm.tile([P, GW], F16, tag=f"vsh{pr}")
            nc.vector.tensor_copy(vsh[:], vs[:])
            st["vsh"] = vsh

        def S_out(g):
            """s2 = s0 + m_b; out = squash(s2); DMA."""
            st = state[g]
            pr = g % 2
            s2 = sm.tile([P, GW], F32, tag=f"s2_{pr}")
            nc.vector.tensor_add(s2[:], st["s0"][:], st["m"][:])
            outt = sm.tile([P, GW], F32, tag=f"outt{pr}")
            squash(g, s2[:], outt[:], "out")
            nc.sync.dma_start(out_d[:, g * GW:(g + 1) * GW], outt[:])

        with nc.allow_low_precision(reason="fp16 routing intermediates"):
            produce(0)
            produce(1)
            for A, B in ((0, 1), (2, 3)):
                S_uv(A, "v0h"); S_otail(A)
                S_uv(B, "v0h"); S_otail(B)
                S_btf(A); S_ut(A); S_itail(A)
                S_btf(B); S_ut(B); S_itail(B)
                S_mid(A); S_uv(A, "vsh"); S_otail(A)
                S_mid(B); S_uv(B, "vsh"); S_otail(B)
                S_btf(A); S_ut(A); S_itail(A)
                if A == 0:
                    produce(2)
                S_btf(B); S_ut(B); S_itail(B)
                if A == 0:
                    produce(3)
                S_out(A)
                S_out(B)

    nc.compile()
    return nc


def _get_program():
    global _PROGRAM
    if _PROGRAM is None:
        _PROGRAM = _build_program()
    return _PROGRAM


def _make_in_maps(inputs):
    x = np.ascontiguousarray(np.asarray(inputs["inputs"], dtype=np.float32))
    W = np.ascontiguousarray(np.asarray(inputs["W"], dtype=np.float32))
    assert x.shape == (16, 8, 8, 128, 16) and W.shape == (32, 128, 16, 16)

    # xt rows: (i%4)*32 + d, cols: (i//4)*128 + p  (d padded 16->32)
    xs = x.reshape(N_CORES, P, I, D)  # [core, p, i, d]
    xt = np.zeros((N_CORES, 4, D2, 32, P), np.float32)
    # [core, i4, d, c, p] <- [core, c, i4, d, p]
    xt[:, :, 0:D] = xs.reshape(N_CORES, P, 32, 4, D).transpose(0, 3, 4, 2, 1)
    xt = xt.reshape(N_CORES, 128, 32 * 128).astype(np.float16)

    # wr rows: (i%4)*32 + d, cols: g*4096 + (i//4)*128 + (k%8)*16 + o
    wv = W.reshape(NG, KC, 32, 4, D, O)  # [g, k8, c, i4, d, o]
    wr = np.zeros((4, D2, NG, 32, KC, O), np.float32)  # [i4, d, g, c, k8, o]
    wr[:, 0:D] = wv.transpose(3, 4, 0, 2, 1, 5)
    wr = np.ascontiguousarray(
        wr.reshape(128, NG * GKO).astype(np.float16)
    )

    return [
        {"xt": np.ascontiguousarray(xt[c]), "wr": wr} for c in range(N_CORES)
    ]


def kernel(**inputs):
    from concourse.bass_utils import run_bass_kernel_spmd

    nc = _get_program()
    in_maps = _make_in_maps(inputs)
    r = run_bass_kernel_spmd(nc, in_maps, list(range(N_CORES)))
    outs = [r.results[c]["out"].reshape(2, 8, 8, K, O) for c in range(N_CORES)]
    return np.concatenate(outs, axis=0).astype(np.float32)


# revision 26
# speedup vs baseline: 1.3034x; 1.1181x over previous
"""CapsLayer2D dynamic-routing kernel for 8x TRN2 NeuronCores.

Problem (hardcoded shapes):
  inputs: [B=16, R=8, C=8, I=128, DIN=16] fp32
  W:      [K=32, I=128, DIN=16, DOUT=16] fp32
  out:    [B, R, C, K, DOUT] fp32

Math (3-round dynamic routing, closed form, verified vs reference):
  U[p,k]    = res[p,k,:,:]  (I x O per position p=(b,r,c) and k)
  s0        = mean_i U_i ; v0 = squash(s0)
  t_a = U v0 ; m_a = U^T t_a ; s1 = s0 + m_a ; v1 = squash(s1)
  t_b = U (v0+v1) ; m_b = U^T t_b ; s2 = s0 + m_b ; out = squash(s2)

Sharding: batch across 8 cores (128 positions/core), W replicated.

Performance design (v5):
  - All W/X layout work (pad d 16->32, transpose to matmul operand
    layout, fp32->fp16 cast) is host-side numpy: zero device prep.
  - 4 k-groups of 8 caps. Production per group: s0 via 32 accumulating
    matmuls; res via 128 per-i matmuls (tile_position quadrants), one
    full PSUM bank per matmul (concurrent start/stop groups must not
    share a bank), strided cross-bank evictions on Act.
  - Routing on the DVE with TENSOR_TENSOR only (2x mode: fp16,
    unit-stride innermost; TRN2 has no 4x for two-stream ops, and
    tensor_reduce has no perf modes at all). Contractions are log2
    trees over sliced views, computed in place inside one scratch
    tile. The U^T t contraction reads t through a duplicated-pair
    tile t2[p,k,i,2] built by a single butterfly add (reversed-stride
    operand), keeping every operand's innermost AP packed.
  - Two groups are software-pipelined: the small tree tails run on the
    Pool engine while the DVE works on the other group, so the DVE
    stream stays gap-free.
"""

import sys

import numpy as np

sys.path.insert(0, "/opt/trn_rl_repo")

P, I, D, K, O = 128, 128, 16, 32, 16
D2 = 32  # padded d
ID = I * D  # 2048
KO = K * O  # 512
KC = 8  # k-group size
NG = K // KC  # 4 groups
GW = KC * O  # 128 group output width
GKO = 32 * KC * O  # per-group W cols: 32 chunks x (k8,o16) = 4096
N_CORES = 8
EPS = 1e-7

_PROGRAM = None


def _build_program():
    from contextlib import ExitStack

    import concourse.tile as tile
    from concourse import bacc, mybir

    F32 = mybir.dt.float32
    F16 = mybir.dt.float16
    ADD = mybir.AluOpType.add
    MULT = mybir.AluOpType.mult
    X = mybir.AxisListType.X
    SQRT = mybir.ActivationFunctionType.Sqrt

    nc = bacc.Bacc("TRN2", target_bir_lowering=False, debug=False)

    xt_d = nc.dram_tensor("xt", [128, 32 * 128], F16, kind="ExternalInput").ap()
    wr_d = nc.dram_tensor("wr", [128, NG * GKO], F16, kind="ExternalInput").ap()
    out_d = nc.dram_tensor("out", [P, KO], F32, kind="ExternalOutput").ap()

    with ExitStack() as ctx:
        tc = ctx.enter_context(tile.TileContext(nc))

        pp_s = ctx.enter_context(tc.tile_pool(name="pp_s", bufs=2, space="PSUM"))
        pp_r = ctx.enter_context(tc.tile_pool(name="pp_r", bufs=2, space="PSUM"))

        xp = ctx.enter_context(tc.tile_pool(name="xt", bufs=1))
        wp = ctx.enter_context(tc.tile_pool(name="wr", bufs=1))
        rp = ctx.enter_context(tc.tile_pool(name="res", bufs=3))
        sp = ctx.enter_context(tc.tile_pool(name="scratch", bufs=1))
        sm = ctx.enter_context(tc.tile_pool(name="small", bufs=1))

        Xt = xp.tile([128, 32 * 128], F16)
        for q in range(4):
            nc.sync.dma_start(
                Xt[:, q * 1024:(q + 1) * 1024],
                xt_d[:, q * 1024:(q + 1) * 1024],
            )

        eps_t = sm.tile([P, 1], F32, tag="eps")
        nc.vector.memset(eps_t[:], EPS)

        state = {g: {} for g in range(NG)}

        def squash(g, s_ap, v_ap, tag):
            """v = squash(s); fp32 [P, (k8,o16)]; sqrt on Act."""
            pr = g % 2
            ssq = sm.tile([P, GW], F32, tag=f"ssq{pr}")
            nc.vector.tensor_mul(ssq[:], s_ap, s_ap)
            sq = sm.tile([P, KC], F32, tag=f"sq{pr}_{tag}")
            nc.vector.tensor_reduce(
                sq[:], ssq[:].rearrange("p (k o) -> p k o", k=KC), X, ADD
            )
            a = sm.tile([P, KC], F32, tag=f"sqa{pr}")
            nc.scalar.activation(a[:], sq[:], SQRT, bias=eps_t[:])
            b = sm.tile([P, KC], F32, tag=f"sqb{pr}")
            nc.vector.scalar_tensor_tensor(b[:], sq[:], 1.0, a[:], ADD, MULT)
            r = sm.tile([P, KC], F32, tag=f"sqr{pr}")
            nc.vector.reciprocal(r[:], b[:])
            f = sm.tile([P, KC], F32, tag=f"sqf{pr}")
            nc.vector.tensor_mul(f[:], sq[:], r[:])
            nc.vector.tensor_mul(
                v_ap.rearrange("p (k o) -> p k o", k=KC),
                s_ap.rearrange("p (k o) -> p k o", k=KC),
                f[:].unsqueeze(2).broadcast_to([P, KC, O]),
            )

        def produce(g):
            st = state[g]
            pr = g % 2
            W_g = wp.tile([128, GKO], F16, tag="wg")
            for q in range(4):
                nc.sync.dma_start(
                    W_g[:, q * 1024:(q + 1) * 1024],
                    wr_d[:, g * GKO + q * 1024:g * GKO + (q + 1) * 1024],
                )

            res = rp.tile([P, KC * I * O], F16, tag="res")
            resv = res[:].rearrange("p (k i o) -> p k i o", k=KC, i=I, o=O)
            for c in range(32):
                for m in range(2):
                    prb = pp_r.tile([P, 1024], F32, tag="prb")
                    for j in (2 * m, 2 * m + 1):
                        r0 = j * 32
                        nc.tensor.matmul(
                            prb[:, (j % 2) * 512:(j % 2) * 512 + GW],
                            Xt[r0:r0 + 32, c * 128:(c + 1) * 128],
                            W_g[r0:r0 + 32, c * 128:(c + 1) * 128],
                            start=True,
                            stop=True,
                            tile_position=(r0, 0),
                        )
                    src = prb[:].rearrange("p (i x) -> p i x", i=2)[
                        :, :, 0:GW
                    ].rearrange("p i (k o) -> p i k o", k=KC)
                    dst = resv[
                        :, :, 4 * c + 2 * m:4 * c + 2 * m + 2, :
                    ].transpose([0, 2, 1, 3])
                    # groups 0/1: DVE is idle before its first work, so
                    # splitting evictions shortens the startup path
                    if g == 0 and m == 1:
                        nc.vector.tensor_copy(dst, src)
                    else:
                        nc.scalar.copy(dst, src)

            # s0 after res: the PE runs res matmuls first so the DVE's
            # first evictions/routing unblock as early as possible
            ps0 = pp_s.tile([P, 512], F32, tag="ps0")
            for c in range(32):
                nc.tensor.matmul(
                    ps0[:, 0:GW],
                    Xt[:, c * 128:(c + 1) * 128],
                    W_g[:, c * 128:(c + 1) * 128],
                    start=(c == 0),
                    stop=(c == 31),
                )
            s0 = sm.tile([P, GW], F32, tag=f"s0_{pr}")
            nc.scalar.mul(s0[:], ps0[:, 0:GW], 1.0 / I)
            v0 = sm.tile([P, GW], F32, tag=f"v0_{pr}")
            squash(g, s0[:], v0[:], "v0")
            v0h = sm.tile([P, GW], F16, tag=f"v0h{pr}")
            nc.vector.tensor_copy(v0h[:], v0[:])

            st["res"] = res
            st["s0"] = s0
            st["v0"] = v0
            st["v0h"] = v0h

        def S_uv(g, vkey):
            """DVE: tmp = res * v (bcast over i); in-place r8 step."""
            st = state[g]
            tmp = sp.tile([P, KC * I * O], F16, tag=f"tmp{g % 2}")
            st["tmp"] = tmp
            t4 = tmp[:].rearrange("p (k i o) -> p k i o", k=KC, i=I)
            rv4 = st["res"][:].rearrange("p (k i o) -> p k i o", k=KC, i=I)
            vb4 = (
                st[vkey][:]
                .rearrange("p (k o) -> p k o", k=KC)
                .unsqueeze(2)
                .broadcast_to([P, KC, I, O])
            )
            nc.vector.tensor_mul(t4, rv4, vb4)
            nc.vector.tensor_add(
                t4[:, :, :, 0:8], t4[:, :, :, 0:8], t4[:, :, :, 8:16]
            )
            nc.vector.tensor_add(
                t4[:, :, :, 0:4], t4[:, :, :, 0:4], t4[:, :, :, 4:8]
            )

        def S_otail(g):
            """DVE: in-place o-tree 4 -> 2."""
            t4 = state[g]["tmp"][:].rearrange("p (k i o) -> p k i o", k=KC, i=I)
            nc.vector.tensor_add(
                t4[:, :, :, 0:2], t4[:, :, :, 0:2], t4[:, :, :, 2:4]
            )

        def S_btf(g):
            """DVE: butterfly -> both t2 slots get the o-pair sum."""
            st = state[g]
            t2 = sp.tile([P, KC * I * 2], F16, tag=f"t2{g % 2}")
            st["t2"] = t2
            r2v = (
                state[g]["tmp"][:]
                .rearrange("p (k i o) -> p k i o", k=KC, i=I)[:, :, :, 0:2]
            )
            t2v = t2[:].rearrange("p (k i two) -> p k i two", k=KC, i=I)
            nc.vector.tensor_add(t2v, r2v, r2v[:, :, :, ::-1])

        def S_ut(g):
            """DVE: tmp = res * t2-pairs; in-place i-tree 128 -> 32."""
            st = state[g]
            tmp = sp.tile([P, KC * I * O], F16, tag=f"tmp{g % 2}")
            st["tmp"] = tmp
            t5 = tmp[:].rearrange(
                "p (k i o2 two) -> p k i o2 two", k=KC, i=I, o2=O // 2
            )
            rv5 = st["res"][:].rearrange(
                "p (k i o2 two) -> p k i o2 two", k=KC, i=I, o2=O // 2
            )
            tb5 = (
                st["t2"][:]
                .rearrange("p (k i two) -> p k i two", k=KC, i=I)
                .unsqueeze(3)
                .broadcast_to([P, KC, I, O // 2, 2])
            )
            nc.vector.tensor_mul(t5, rv5, tb5)
            t4 = tmp[:].rearrange("p (k i o) -> p k i o", k=KC, i=I)
            nc.vector.tensor_add(
                t4[:, :, 0:64, :], t4[:, :, 0:64, :], t4[:, :, 64:128, :]
            )
            nc.vector.tensor_add(
                t4[:, :, 0:32, :], t4[:, :, 0:32, :], t4[:, :, 32:64, :]
            )
            nc.vector.tensor_add(
                t4[:, :, 0:16, :], t4[:, :, 0:16, :], t4[:, :, 16:32, :]
            )

        def S_itail(g):
            """DVE: in-place i-tree 16 -> 2, then m = row0 + row1."""
            st = state[g]
            t4 = st["tmp"][:].rearrange("p (k i o) -> p k i o", k=KC, i=I)
            n = 16
            while n > 2:
                h = n // 2
                nc.vector.tensor_add(
                    t4[:, :, 0:h, :], t4[:, :, 0:h, :], t4[:, :, h:n, :]
                )
                n = h
            m_t = sm.tile([P, GW], F16, tag=f"m{g % 2}")
            st["m"] = m_t
            nc.vector.tensor_add(
                m_t[:].rearrange("p (k o) -> p k o", k=KC),
                t4[:, :, 0, :],
                t4[:, :, 1, :],
            )

        def S_mid(g):
            """s1 = s0 + m_a; v1 = squash(s1); vsh = fp16(v0 + v1)."""
            st = state[g]
            pr = g % 2
            s1 = sm.tile([P, GW], F32, tag=f"s1_{pr}")
            nc.vector.tensor_add(s1[:], st["s0"][:], st["m"][:])
            v1 = sm.tile([P, GW], F32, tag=f"v1_{pr}")
            squash(g, s1[:], v1[:], "v1")
            vs = sm.tile([P, GW], F32, tag=f"vs{pr}")
            nc.vector.tensor_add(vs[:], st["v0"][:], v1[:])
            vsh = sm.tile([P, GW], F16, tag=f"vsh{pr}")
            nc.vector.tensor_copy(vsh[:], vs[:])
            st["vsh"] = vsh

        def S_out(g):
            """s2 = s0 + m_b; out = squash(s2); DMA."""
            st = state[g]
            pr = g % 2
            s2 = sm.tile([P, GW], F32, tag=f"s2_{pr}")
            nc.vector.tensor_add(s2[:], st["s0"][:], st["m"][:])
            outt = sm.tile([P, GW], F32, tag=f"outt{pr}")
            squash(g, s2[:], outt[:], "out")
            nc.sync.dma_start(out_d[:, g * GW:(g + 1) * GW], outt[:])

        with nc.allow_low_precision(reason="fp16 routing intermediates"):
            produce(0)
            produce(1)
            for A, B in ((0, 1), (2, 3)):
                S_uv(A, "v0h"); S_otail(A)
                S_uv(B, "v0h"); S_otail(B)
                S_btf(A); S_ut(A); S_itail(A)
                S_btf(B); S_ut(B); S_itail(B)
                S_mid(A); S_uv(A, "vsh"); S_otail(A)
                S_mid(B); S_uv(B, "vsh"); S_otail(B)
                S_btf(A); S_ut(A); S_itail(A)
                if A == 0:
                    produce(2)
                S_btf(B); S_ut(B); S_itail(B)
                if A == 0:
                    produce(3)
                S_out(A)
                S_out(B)

    nc.compile()
    return nc


def _get_program():
    global _PROGRAM
    if _PROGRAM is None:
        _PROGRAM = _build_program()
    return _PROGRAM


def _make_in_maps(inputs):
    x = np.ascontiguousarray(np.asarray(inputs["inputs"], dtype=np.float32))
    W = np.ascontiguousarray(np.asarray(inputs["W"], dtype=np.float32))
    assert x.shape == (16, 8, 8, 128, 16) and W.shape == (32, 128, 16, 16)

    # xt rows: (i%4)*32 + d, cols: (i//4)*128 + p  (d padded 16->32)
    xs = x.reshape(N_CORES, P, I, D)  # [core, p, i, d]
    xt = np.zeros((N_CORES, 4, D2, 32, P), np.float32)
    # [core, i4, d, c, p] <- [core, c, i4, d, p]
    xt[:, :, 0:D] = xs.reshape(N_CORES, P, 32, 4, D).transpose(0, 3, 4, 2, 1)
    xt = xt.reshape(N_CORES, 128, 32 * 128).astype(np.float16)

    # wr rows: (i%4)*32 + d, cols: g*4096 + (i//4)*128 + (k%8)*16 + o
    wv = W.reshape(NG, KC, 32, 4, D, O)  # [g, k8, c, i4, d, o]
    wr = np.zeros((4, D2, NG, 32, KC, O), np.float32)  # [i4, d, g, c, k8, o]
    wr[:, 0:D] = wv.transpose(3, 4, 0, 2, 1, 5)
    wr = np.ascontiguousarray(
        wr.reshape(128, NG * GKO).astype(np.float16)
    )

    return [
        {"xt": np.ascontiguousarray(xt[c]), "wr": wr} for c in range(N_CORES)
    ]


def kernel(**inputs):
    from concourse.bass_utils import run_bass_kernel_spmd

    nc = _get_program()
    in_maps = _make_in_maps(inputs)
    r = run_bass_kernel_spmd(nc, in_maps, list(range(N_CORES)))
    outs = [r.results[c]["out"].reshape(2, 8, 8, K, O) for c in range(N_CORES)]
    return np.concatenate(outs, axis=0).astype(np.float32)
